# revision 4
# baseline (speedup 1.0000x reference)
"""GQA attention block (B=2,S=1024,H=4096, 32 q-heads / 8 kv-heads, RoPE, causal)
on 8 trn2 NeuronCores.

Sharding: tensor-parallel over heads. Core d owns kv-head d and q-heads
4d..4d+3. Each core computes its 4 attention heads end-to-end plus the partial
dense projection out_part.T = wd[:, cols_d] @ ctx_d.T; the host sums the 8
fp32 partials (gather is host-side anyway, so no device collective is needed).

All matmuls run in float32r (full PE rate, ~1.4e-4 rel err vs fp32).
Layout on chip is feature-major ("transposed"): activations hsT [H, n],
Q/K as [hd, n] so the PE contracts over partitions everywhere; host does all
the cheap numpy transposes.
"""
import math
import numpy as np

import concourse.bass as bass
import concourse.tile as tile
from concourse import bacc, mybir
from concourse.bass_utils import run_bass_kernel_spmd

B, S, H = 2, 1024, 4096
NH, NKV, HD = 32, 8, 128
GROUP = NH // NKV          # q-heads per kv-head = per-core q-heads
NT = B * S                 # 2048 tokens
QD = GROUP * HD            # 512 per-core q features
NCH = NT // 512            # 4 token chunks of 512
HCH = H // 128             # 32 contraction chunks

f32 = mybir.dt.float32
f32r = mybir.dt.float32r
AF = mybir.ActivationFunctionType

_CACHE = {}


def _build_nc():
    nc = bacc.Bacc("TRN2", target_bir_lowering=False, debug=False, num_devices=8)
    io = {}
    io["hsT"] = nc.dram_tensor("hsT", [H, NT], f32r, kind="ExternalInput").ap()
    io["wqT"] = nc.dram_tensor("wqT", [H, QD], f32r, kind="ExternalInput").ap()
    io["wkT"] = nc.dram_tensor("wkT", [H, HD], f32r, kind="ExternalInput").ap()
    io["wvT"] = nc.dram_tensor("wvT", [H, HD], f32r, kind="ExternalInput").ap()
    io["wdT"] = nc.dram_tensor("wdT", [QD, H], f32r, kind="ExternalInput").ap()
    io["cosT"] = nc.dram_tensor("cosT", [HD, S], f32, kind="ExternalInput").ap()
    io["sinTs"] = nc.dram_tensor("sinTs", [HD, S], f32, kind="ExternalInput").ap()
    io["mask01"] = nc.dram_tensor("mask01", [128, 640], f32, kind="ExternalInput").ap()
    io["ident"] = nc.dram_tensor("ident", [128, 128], f32, kind="ExternalInput").ap()
    io["ones"] = nc.dram_tensor("ones", [128, 1], f32r, kind="ExternalInput").ap()
    io["outT"] = nc.dram_tensor("outT", [H, NT], f32, kind="ExternalOutput").ap()
    io["kT"] = nc.dram_tensor("kT", [HD, NT], f32, kind="ExternalOutput").ap()
    io["vT"] = nc.dram_tensor("vT", [HD, NT], f32, kind="ExternalOutput").ap()

    with tile.TileContext(nc) as tc:
        _emit(nc, tc, io)
    nc.compile()
    return nc


def _emit(nc, tc, io):
    from contextlib import ExitStack

    ctx = ExitStack()
    with ctx:
        const_p = ctx.enter_context(tc.tile_pool(name="const", bufs=1))
        wkv_p = ctx.enter_context(tc.tile_pool(name="wkv", bufs=1))
        persist_p = ctx.enter_context(tc.tile_pool(name="persist", bufs=1))
        hs_p = ctx.enter_context(tc.tile_pool(name="hs", bufs=4))
        wq_p = ctx.enter_context(tc.tile_pool(name="wq", bufs=4))
        rope_p = ctx.enter_context(tc.tile_pool(name="rope", bufs=2))
        exp_p = ctx.enter_context(tc.tile_pool(name="exp", bufs=2))
        nrm_p = ctx.enter_context(tc.tile_pool(name="nrm", bufs=1))
        wd_p = ctx.enter_context(tc.tile_pool(name="wd", bufs=6))
        osb_p = ctx.enter_context(tc.tile_pool(name="osb", bufs=2))
        # ---- constants / small tables ----
        cos_sb = const_p.tile([HD, S], f32, tag="cos", name="cos")
        nc.sync.dma_start(cos_sb[:], io["cosT"])
        sin_sb = const_p.tile([HD, S], f32, tag="sin", name="sin")
        nc.sync.dma_start(sin_sb[:], io["sinTs"])
        mask_sb = const_p.tile([128, 640], f32, tag="mask", name="mask")
        nc.sync.dma_start(mask_sb[:], io["mask01"])
        ident_sb = const_p.tile([128, 128], f32, tag="ident", name="ident")
        nc.sync.dma_start(ident_sb[:], io["ident"])
        ones_sb = const_p.tile([128, 1], f32r, tag="ones", name="ones")
        nc.sync.dma_start(ones_sb[:], io["ones"])

        # ---- resident K/V weights ----
        wk_sb = wkv_p.tile([128, HCH, HD], f32r, tag="wk", name="wk")
        nc.sync.dma_start(wk_sb[:], io["wkT"].rearrange("(c p) d -> p c d", p=128))
        wv_sb = wkv_p.tile([128, HCH, HD], f32r, tag="wv", name="wv")
        nc.sync.dma_start(wv_sb[:], io["wvT"].rearrange("(c p) d -> p c d", p=128))

        # ---- persistent activations ----
        qt = [persist_p.tile([128, NT], f32r, tag=f"qt{j}", name=f"qt{j}") for j in range(GROUP)]
        ktr = persist_p.tile([128, NT], f32r, tag="ktr", name="ktr")     # RoPE'd K, f32r
        vnat = [persist_p.tile([128, 128], f32r, tag=f"vn{i}", name=f"vn{i}") for i in range(NT // 128)]
        ctxT = [persist_p.tile([128, NT], f32r, tag=f"cx{j}", name=f"cx{j}") for j in range(GROUP)]

        # ================= stage A: QKV projections + RoPE =================
        ps_a_ctx = ExitStack()
        ps_a = ps_a_ctx.enter_context(tc.tile_pool(name="ps_a", bufs=1, space="PSUM"))
        for c in range(NCH):
            n0 = c * 512
            qps = [ps_a.tile([128, 512], f32, tag=f"qps{j}", name=f"qps{j}") for j in range(GROUP)]
            kps = ps_a.tile([128, 512], f32, tag="kps", name="kps")
            vps = ps_a.tile([128, 512], f32, tag="vps", name="vps")
            for h in range(HCH):
                hs_t = hs_p.tile([128, 512], f32r, tag="hs", name="hs")
                nc.sync.dma_start(hs_t[:], io["hsT"][h * 128:(h + 1) * 128, n0:n0 + 512])
                wq_t = wq_p.tile([128, QD], f32r, tag="wq", name="wq")
                nc.sync.dma_start(wq_t[:], io["wqT"][h * 128:(h + 1) * 128, :])
                st = (h == 0)
                sp = (h == HCH - 1)
                for j in range(GROUP):
                    nc.tensor.matmul(qps[j][:], wq_t[:, j * 128:(j + 1) * 128], hs_t[:],
                                     start=st, stop=sp)
                nc.tensor.matmul(kps[:], wk_sb[:, h, :], hs_t[:], start=st, stop=sp)
                nc.tensor.matmul(vps[:], wv_sb[:, h, :], hs_t[:], start=st, stop=sp)

            off = (c % (S // 512)) * 512  # position offset within batch
            cos_sl = cos_sb[:, off:off + 512]
            sin_sl = sin_sb[:, off:off + 512]
            # Q RoPE -> qt[j] (f32r)
            for j in range(GROUP):
                rq = rope_p.tile([128, 512], f32, tag="rq", name="rq")
                nc.vector.tensor_copy(rq[0:64, :], qps[j][64:128, :])
                nc.vector.tensor_copy(rq[64:128, :], qps[j][0:64, :])
                nc.vector.tensor_mul(rq[:], rq[:], sin_sl)
                qc_t = rope_p.tile([128, 512], f32, tag="qcos", name="qcos")
                nc.vector.tensor_mul(qc_t[:], qps[j][:], cos_sl)
                nc.vector.tensor_add(qt[j][:, n0:n0 + 512], qc_t[:], rq[:])
            # K RoPE -> kt_out (f32, staged for DMA) + ktr (f32r)
            rk = rope_p.tile([128, 512], f32, tag="rk", name="rk")
            nc.vector.tensor_copy(rk[0:64, :], kps[64:128, :])
            nc.vector.tensor_copy(rk[64:128, :], kps[0:64, :])
            nc.vector.tensor_mul(rk[:], rk[:], sin_sl)
            kt_out = rope_p.tile([128, 512], f32, tag="kcos", name="kcos")
            nc.vector.tensor_mul(kt_out[:], kps[:], cos_sl)
            nc.vector.tensor_add(kt_out[:], kt_out[:], rk[:])
            nc.scalar.copy(ktr[:, n0:n0 + 512], kt_out[:])
            nc.sync.dma_start(io["kT"][:, n0:n0 + 512], kt_out[:])
            # V -> staging (f32) + output
            vt_out = rope_p.tile([128, 512], f32, tag="vstage", name="vstage")
            nc.scalar.copy(vt_out[:], vps[:])
            nc.sync.dma_start(io["vT"][:, n0:n0 + 512], vt_out[:])
            # V transpose -> vnat (f32r), 4 tiles of 128
            for i in range(4):
                g = c * 4 + i
                tp = ps_a.tile([128, 128], f32, tag="tp", name="tp")
                nc.tensor.transpose(tp[:], vt_out[:, i * 128:(i + 1) * 128], ident_sb[:])
                nc.vector.tensor_copy(vnat[g][:], tp[:])

        ps_a_ctx.close()
        # ================= stage B: attention per (batch, q-head) =================
        ps_b_ctx = ExitStack()
        ps_b = ps_b_ctx.enter_context(tc.tile_pool(name="ps_b", bufs=1, space="PSUM"))
        for b in range(B):
            base = b * S
            for j in range(GROUP):
                cps = ps_b.tile([128, S], f32, tag="cps", name="cps")    # ctxT_un [hd, q]
                dps = ps_b.tile([1, S], f32, tag="dps", name="dps")      # denominators [1, q]
                for t in range(S // 128):                    # key chunks
                    qcd = t // 4                             # q-chunk holding the diagonal
                    sps = ps_b.tile([128, S], f32, tag="sps", name="sps")
                    et = exp_p.tile([128, S], f32r, tag="et", name="et")
                    for qc in range(S // 512):
                        if qc < qcd:
                            continue                          # fully masked
                        q0 = qc * 512
                        nc.tensor.matmul(sps[:, q0:q0 + 512],
                                         ktr[:, base + t * 128:base + (t + 1) * 128],
                                         qt[j][:, base + q0:base + q0 + 512])
                        nc.scalar.activation(et[:, q0:q0 + 512], sps[:, q0:q0 + 512], AF.Exp)
                    # zero the masked region: columns [qcd*512 .. t*128+128)
                    o = t * 128 - qcd * 512
                    nc.vector.tensor_mul(et[:, qcd * 512:t * 128 + 128],
                                         et[:, qcd * 512:t * 128 + 128].bitcast(f32),
                                         mask_sb[:, 512 - o:640])
                    for qc in range(S // 512):
                        if qc < qcd:
                            continue
                        q0 = qc * 512
                        st = (t == 0)
                        sp = (t == (qc * 4 + 3))
                        nc.tensor.matmul(cps[:, q0:q0 + 512], vnat[b * 8 + t][:],
                                         et[:, q0:q0 + 512], start=st, stop=sp)
                        nc.tensor.matmul(dps[:, q0:q0 + 512], ones_sb[:],
                                         et[:, q0:q0 + 512], start=st, stop=sp)
                rec = nrm_p.tile([1, S], f32, tag="rec", name="rec")
                nc.vector.reciprocal(rec[:], dps[:])
                bc = nrm_p.tile([128, S], f32, tag="bc", name="bc")
                nc.gpsimd.partition_broadcast(bc[:], rec[:])
                nc.vector.tensor_mul(ctxT[j][:, base:base + S], cps[:], bc[:])

        ps_b_ctx.close()
        # ================= stage C: dense partial out.T = wd_d @ ctx_d.T =============
        ps_c = ctx.enter_context(tc.tile_pool(name="ps_c", bufs=1, space="PSUM"))
        for cp in range(2):                                  # batch halves (n chunks 2cp, 2cp+1)
            for ho in range(H // 128):
                ops = [ps_c.tile([128, 512], f32, tag=f"ops{i}", name=f"ops{i}") for i in range(2)]
                for ci in range(GROUP):
                    wd_t = wd_p.tile([128, 128], f32r, tag="wd", name="wd")
                    nc.sync.dma_start(
                        wd_t[:], io["wdT"][ci * 128:(ci + 1) * 128, ho * 128:(ho + 1) * 128])
                    for i in range(2):
                        n0 = (2 * cp + i) * 512
                        nc.tensor.matmul(ops[i][:], wd_t[:], ctxT[ci][:, n0:n0 + 512],
                                         start=(ci == 0), stop=(ci == GROUP - 1))
                for i in range(2):
                    n0 = (2 * cp + i) * 512
                    ot = osb_p.tile([128, 512], f32, tag="ot", name="ot")
                    nc.scalar.copy(ot[:], ops[i][:])
                    nc.sync.dma_start(io["outT"][ho * 128:(ho + 1) * 128, n0:n0 + 512], ot[:])


def _host_inputs(hidden_states, cos, sin, wq, wk, wv, wd):
    """Build the 8 per-core input maps (all numpy, fp32)."""
    hs = np.asarray(hidden_states, dtype=np.float32).reshape(NT, H)
    hsT = np.ascontiguousarray(hs.T)
    scale = np.float32(1.0 / math.sqrt(HD))
    wq = np.asarray(wq, dtype=np.float32)
    wk = np.asarray(wk, dtype=np.float32)
    wv = np.asarray(wv, dtype=np.float32)
    wd = np.asarray(wd, dtype=np.float32)
    wqsT = np.ascontiguousarray((wq * scale).T)        # [H, NH*HD]
    wkT = np.ascontiguousarray(wk.T)                   # [H, NKV*HD]
    wvT = np.ascontiguousarray(wv.T)
    wdT = np.ascontiguousarray(wd.T)                   # [H_in(ctx), H_out]

    cosT = np.ascontiguousarray(np.asarray(cos, dtype=np.float32)[0, 0].T)  # [HD, S]
    sinT = np.ascontiguousarray(np.asarray(sin, dtype=np.float32)[0, 0].T)
    sinTs = sinT.copy()
    sinTs[0:HD // 2] = -sinT[0:HD // 2]

    mask01 = np.zeros((128, 640), np.float32)
    tri = (np.arange(128)[:, None] <= np.arange(128)[None, :])
    mask01[:, 512:640] = tri.astype(np.float32)
    ident = np.eye(128, dtype=np.float32)
    ones = np.ones((128, 1), np.float32)

    in_maps = []
    for d in range(8):
        in_maps.append({
            "hsT": hsT,
            "wqT": np.ascontiguousarray(wqsT[:, d * QD:(d + 1) * QD]),
            "wkT": np.ascontiguousarray(wkT[:, d * HD:(d + 1) * HD]),
            "wvT": np.ascontiguousarray(wvT[:, d * HD:(d + 1) * HD]),
            "wdT": np.ascontiguousarray(wdT[d * QD:(d + 1) * QD, :]),
            "cosT": cosT,
            "sinTs": sinTs,
            "mask01": mask01,
            "ident": ident,
            "ones": ones,
        })
    return in_maps


def _gather(results):
    outT = np.zeros((H, NT), np.float64)
    for d in range(8):
        outT += results[d]["outT"]
    out = outT.T.astype(np.float32).reshape(B, S, H)
    k = np.stack([results[d]["kT"].reshape(HD, B, S).transpose(1, 2, 0)
                  for d in range(8)], axis=1)             # [B, NKV, S, HD]
    v = np.stack([results[d]["vT"].reshape(HD, B, S).transpose(1, 2, 0)
                  for d in range(8)], axis=1)
    return out, np.ascontiguousarray(k), np.ascontiguousarray(v)


def _run(in_maps, **kwargs):
    if "nc" not in _CACHE:
        _CACHE["nc"] = _build_nc()
    return run_bass_kernel_spmd(_CACHE["nc"], in_maps, list(range(8)), **kwargs)


def kernel(hidden_states, cos, sin, wq, wk, wv, wd):
    in_maps = _host_inputs(hidden_states, cos, sin, wq, wk, wv, wd)
    res = _run(in_maps)
    return _gather(res.results)


# revision 5
# speedup vs baseline: 1.0710x; 1.0710x over previous
"""GQA attention block (B=2,S=1024,H=4096, 32 q-heads / 8 kv-heads, RoPE, causal)
on 8 trn2 NeuronCores.

Sharding: tensor-parallel over heads. Core d owns kv-head d and q-heads
4d..4d+3. Each core computes its 4 attention heads end-to-end plus the partial
dense projection out_part.T = wd[:, cols_d] @ ctx_d.T; the host sums the 8
fp32 partials (gather is host-side anyway, so no device collective is needed).

All matmuls run in float32r (full PE rate, ~1.4e-4 rel err vs fp32).
Layout on chip is feature-major ("transposed"): activations hsT [H, n],
Q/K as [hd, n] so the PE contracts over partitions everywhere; host does all
the cheap numpy transposes.
"""
import math
import numpy as np

import concourse.bass as bass
import concourse.tile as tile
from concourse import bacc, mybir
from concourse.bass_utils import run_bass_kernel_spmd

B, S, H = 2, 1024, 4096
NH, NKV, HD = 32, 8, 128
GROUP = NH // NKV          # q-heads per kv-head = per-core q-heads
NT = B * S                 # 2048 tokens
QD = GROUP * HD            # 512 per-core q features
NCH = NT // 512            # 4 token chunks of 512
HCH = H // 128             # 32 contraction chunks

f32 = mybir.dt.float32
f32r = mybir.dt.float32r
AF = mybir.ActivationFunctionType

_CACHE = {}


def _build_nc():
    nc = bacc.Bacc("TRN2", target_bir_lowering=False, debug=False, num_devices=8)
    io = {}
    io["hsT"] = nc.dram_tensor("hsT", [H, NT], f32r, kind="ExternalInput").ap()
    io["wqT"] = nc.dram_tensor("wqT", [H, QD], f32r, kind="ExternalInput").ap()
    io["wkT"] = nc.dram_tensor("wkT", [H, HD], f32r, kind="ExternalInput").ap()
    io["wvT"] = nc.dram_tensor("wvT", [H, HD], f32r, kind="ExternalInput").ap()
    io["wdT"] = nc.dram_tensor("wdT", [QD, H], f32r, kind="ExternalInput").ap()
    io["cosT"] = nc.dram_tensor("cosT", [HD, S], f32, kind="ExternalInput").ap()
    io["sinTs"] = nc.dram_tensor("sinTs", [HD, S], f32, kind="ExternalInput").ap()
    io["mask01"] = nc.dram_tensor("mask01", [128, 640], f32, kind="ExternalInput").ap()
    io["ident"] = nc.dram_tensor("ident", [128, 128], f32, kind="ExternalInput").ap()
    io["ones"] = nc.dram_tensor("ones", [128, 1], f32r, kind="ExternalInput").ap()
    io["outT"] = nc.dram_tensor("outT", [H, NT], f32, kind="ExternalOutput").ap()
    io["kT"] = nc.dram_tensor("kT", [HD, NT], f32, kind="ExternalOutput").ap()
    io["vT"] = nc.dram_tensor("vT", [HD, NT], f32, kind="ExternalOutput").ap()

    with tile.TileContext(nc) as tc:
        _emit(nc, tc, io)
    nc.compile()
    return nc


def _emit(nc, tc, io):
    from contextlib import ExitStack

    ctx = ExitStack()
    with ctx:
        const_p = ctx.enter_context(tc.tile_pool(name="const", bufs=1))
        persist_p = ctx.enter_context(tc.tile_pool(name="persist", bufs=1))
        hs_p = ctx.enter_context(tc.tile_pool(name="hs", bufs=2))
        wq_p = ctx.enter_context(tc.tile_pool(name="wq", bufs=2))
        wkv_p = ctx.enter_context(tc.tile_pool(name="wkv", bufs=2))
        rope_p = ctx.enter_context(tc.tile_pool(name="rope", bufs=2))
        exp_p = ctx.enter_context(tc.tile_pool(name="exp", bufs=2))
        nrm_p = ctx.enter_context(tc.tile_pool(name="nrm", bufs=1))
        wd_p = ctx.enter_context(tc.tile_pool(name="wd", bufs=4))
        osb_p = ctx.enter_context(tc.tile_pool(name="osb", bufs=2))

        # ---- constants / small tables ----
        cos_sb = const_p.tile([HD, S], f32, tag="cos", name="cos")
        nc.sync.dma_start(cos_sb[:], io["cosT"])
        sin_sb = const_p.tile([HD, S], f32, tag="sin", name="sin")
        nc.sync.dma_start(sin_sb[:], io["sinTs"])
        mask_sb = const_p.tile([128, 640], f32, tag="mask", name="mask")
        nc.sync.dma_start(mask_sb[:], io["mask01"])
        ident_sb = const_p.tile([128, 128], f32, tag="ident", name="ident")
        nc.sync.dma_start(ident_sb[:], io["ident"])
        ones_sb = const_p.tile([128, 1], f32r, tag="ones", name="ones")
        nc.sync.dma_start(ones_sb[:], io["ones"])

        # ---- persistent activations ----
        qt = [persist_p.tile([128, NT], f32r, tag=f"qt{j}", name=f"qt{j}") for j in range(GROUP)]
        ktr = persist_p.tile([128, NT], f32r, tag="ktr", name="ktr")
        vnat = [persist_p.tile([128, 128], f32r, tag=f"vn{i}", name=f"vn{i}") for i in range(NT // 128)]
        ctxT = [persist_p.tile([128, NT], f32r, tag=f"cx{j}", name=f"cx{j}") for j in range(GROUP)]

        HB = 4          # h-chunks per batched DMA
        hsT_r = io["hsT"].rearrange("(k p) n -> p k n", p=128)   # [128, 32, 2048]
        wqT_r = io["wqT"].rearrange("(k p) d -> p k d", p=128)   # [128, 32, 512]
        wkT_r = io["wkT"].rearrange("(k p) d -> p k d", p=128)   # [128, 32, 128]
        wvT_r = io["wvT"].rearrange("(k p) d -> p k d", p=128)

        # ================= stage A: QKV projections + RoPE =================
        ps_a_ctx = ExitStack()
        ps_a = ps_a_ctx.enter_context(tc.tile_pool(name="ps_a", bufs=1, space="PSUM"))
        for c in range(NCH):
            n0 = c * 512
            qps = [ps_a.tile([128, 512], f32, tag=f"qps{j}", name=f"qps{j}") for j in range(GROUP)]
            kps = ps_a.tile([128, 512], f32, tag="kps", name="kps")
            vps = ps_a.tile([128, 512], f32, tag="vps", name="vps")
            for hb in range(HCH // HB):
                h0 = hb * HB
                hs_t = hs_p.tile([128, HB, 512], f32r, tag="hs", name="hs")
                nc.sync.dma_start(hs_t[:], hsT_r[:, h0:h0 + HB, n0:n0 + 512])
                wq_t = wq_p.tile([128, HB, QD], f32r, tag="wq", name="wq")
                nc.sync.dma_start(wq_t[:], wqT_r[:, h0:h0 + HB, :])
                wk_t = wkv_p.tile([128, HB, HD], f32r, tag="wk", name="wk")
                nc.sync.dma_start(wk_t[:], wkT_r[:, h0:h0 + HB, :])
                wv_t = wkv_p.tile([128, HB, HD], f32r, tag="wv", name="wv")
                nc.sync.dma_start(wv_t[:], wvT_r[:, h0:h0 + HB, :])
                for i in range(HB):
                    h = h0 + i
                    st = (h == 0)
                    sp = (h == HCH - 1)
                    for j in range(GROUP):
                        nc.tensor.matmul(qps[j][:], wq_t[:, i, j * 128:(j + 1) * 128],
                                         hs_t[:, i, :], start=st, stop=sp)
                    nc.tensor.matmul(kps[:], wk_t[:, i, :], hs_t[:, i, :], start=st, stop=sp)
                    nc.tensor.matmul(vps[:], wv_t[:, i, :], hs_t[:, i, :], start=st, stop=sp)

            off = (c % (S // 512)) * 512  # position offset within batch
            cos_sl = cos_sb[:, off:off + 512]
            sin_sl = sin_sb[:, off:off + 512]
            # Q RoPE -> qt[j] (f32r)
            for j in range(GROUP):
                rq = rope_p.tile([128, 512], f32, tag="rq", name="rq")
                nc.vector.tensor_copy(rq[0:64, :], qps[j][64:128, :])
                nc.vector.tensor_copy(rq[64:128, :], qps[j][0:64, :])
                nc.vector.tensor_mul(rq[:], rq[:], sin_sl)
                qc_t = rope_p.tile([128, 512], f32, tag="qcos", name="qcos")
                nc.vector.tensor_mul(qc_t[:], qps[j][:], cos_sl)
                nc.vector.tensor_add(qt[j][:, n0:n0 + 512], qc_t[:], rq[:])
            # K RoPE -> kt_out (f32, staged for DMA) + ktr (f32r)
            rk = rope_p.tile([128, 512], f32, tag="rk", name="rk")
            nc.vector.tensor_copy(rk[0:64, :], kps[64:128, :])
            nc.vector.tensor_copy(rk[64:128, :], kps[0:64, :])
            nc.vector.tensor_mul(rk[:], rk[:], sin_sl)
            kt_out = rope_p.tile([128, 512], f32, tag="kcos", name="kcos")
            nc.vector.tensor_mul(kt_out[:], kps[:], cos_sl)
            nc.vector.tensor_add(kt_out[:], kt_out[:], rk[:])
            nc.scalar.copy(ktr[:, n0:n0 + 512], kt_out[:])
            nc.sync.dma_start(io["kT"][:, n0:n0 + 512], kt_out[:])
            # V -> staging (f32) + output
            vt_out = rope_p.tile([128, 512], f32, tag="vstage", name="vstage")
            nc.scalar.copy(vt_out[:], vps[:])
            nc.sync.dma_start(io["vT"][:, n0:n0 + 512], vt_out[:])
            # V transpose -> vnat (f32r), 4 tiles of 128
            for i in range(4):
                g = c * 4 + i
                tp = ps_a.tile([128, 128], f32, tag="tp", name="tp")
                nc.tensor.transpose(tp[:], vt_out[:, i * 128:(i + 1) * 128], ident_sb[:])
                nc.vector.tensor_copy(vnat[g][:], tp[:])

        ps_a_ctx.close()
        # ================= stage B: attention per (batch, q-head) =================
        ps_b_ctx = ExitStack()
        ps_b = ps_b_ctx.enter_context(tc.tile_pool(name="ps_b", bufs=1, space="PSUM"))
        for b in range(B):
            base = b * S
            for j in range(GROUP):
                cps = ps_b.tile([128, S], f32, tag="cps", name="cps")    # ctxT_un [hd, q]
                dps = ps_b.tile([1, S], f32, tag="dps", name="dps")      # denominators [1, q]
                for t in range(S // 128):                    # key chunks
                    qcd = t // 4                             # q-chunk holding the diagonal
                    sps = ps_b.tile([128, S], f32, tag="sps", name="sps", bufs=2)
                    et = exp_p.tile([128, S], f32r, tag="et", name="et")
                    for qc in range(S // 512):
                        if qc < qcd:
                            continue                          # fully masked
                        q0 = qc * 512
                        nc.tensor.matmul(sps[:, q0:q0 + 512],
                                         ktr[:, base + t * 128:base + (t + 1) * 128],
                                         qt[j][:, base + q0:base + q0 + 512])
                        nc.scalar.activation(et[:, q0:q0 + 512], sps[:, q0:q0 + 512], AF.Exp)
                    # zero the masked region: columns [qcd*512 .. t*128+128)
                    o = t * 128 - qcd * 512
                    nc.vector.tensor_mul(et[:, qcd * 512:t * 128 + 128],
                                         et[:, qcd * 512:t * 128 + 128].bitcast(f32),
                                         mask_sb[:, 512 - o:640])
                    for qc in range(S // 512):
                        if qc < qcd:
                            continue
                        q0 = qc * 512
                        st = (t == 0)
                        sp = (t == (qc * 4 + 3))
                        nc.tensor.matmul(cps[:, q0:q0 + 512], vnat[b * 8 + t][:],
                                         et[:, q0:q0 + 512], start=st, stop=sp)
                        nc.tensor.matmul(dps[:, q0:q0 + 512], ones_sb[:],
                                         et[:, q0:q0 + 512], start=st, stop=sp)
                rec = nrm_p.tile([1, S], f32, tag="rec", name="rec")
                nc.vector.reciprocal(rec[:], dps[:])
                bc = nrm_p.tile([128, S], f32, tag="bc", name="bc")
                nc.gpsimd.partition_broadcast(bc[:], rec[:])
                nc.vector.tensor_mul(ctxT[j][:, base:base + S], cps[:], bc[:])

        ps_b_ctx.close()
        # ================= stage C: dense partial out.T = wd_d @ ctx_d.T =============
        ps_c = ctx.enter_context(tc.tile_pool(name="ps_c", bufs=1, space="PSUM"))
        wdT_r = io["wdT"].rearrange("(k p) n -> p k n", p=128)   # [128, 4, 4096]
        for cp in range(2):                                  # batch halves (n chunks 2cp, 2cp+1)
            for ho in range(H // 128):
                ops = [ps_c.tile([128, 512], f32, tag=f"ops{i}", name=f"ops{i}", bufs=2)
                       for i in range(2)]
                wd_t = wd_p.tile([128, GROUP, 128], f32r, tag="wd", name="wd")
                nc.sync.dma_start(wd_t[:], wdT_r[:, :, ho * 128:(ho + 1) * 128])
                for ci in range(GROUP):
                    for i in range(2):
                        n0 = (2 * cp + i) * 512
                        nc.tensor.matmul(ops[i][:], wd_t[:, ci, :], ctxT[ci][:, n0:n0 + 512],
                                         start=(ci == 0), stop=(ci == GROUP - 1))
                for i in range(2):
                    n0 = (2 * cp + i) * 512
                    ot = osb_p.tile([128, 512], f32, tag="ot", name="ot")
                    if i == 0:
                        nc.scalar.copy(ot[:], ops[i][:])
                    else:
                        nc.vector.tensor_copy(ot[:], ops[i][:])
                    nc.sync.dma_start(io["outT"][ho * 128:(ho + 1) * 128, n0:n0 + 512], ot[:])


def _host_inputs(hidden_states, cos, sin, wq, wk, wv, wd):
    """Build the 8 per-core input maps (all numpy, fp32)."""
    hs = np.asarray(hidden_states, dtype=np.float32).reshape(NT, H)
    hsT = np.ascontiguousarray(hs.T)
    scale = np.float32(1.0 / math.sqrt(HD))
    wq = np.asarray(wq, dtype=np.float32)
    wk = np.asarray(wk, dtype=np.float32)
    wv = np.asarray(wv, dtype=np.float32)
    wd = np.asarray(wd, dtype=np.float32)
    wqsT = np.ascontiguousarray((wq * scale).T)        # [H, NH*HD]
    wkT = np.ascontiguousarray(wk.T)                   # [H, NKV*HD]
    wvT = np.ascontiguousarray(wv.T)
    wdT = np.ascontiguousarray(wd.T)                   # [H_in(ctx), H_out]

    cosT = np.ascontiguousarray(np.asarray(cos, dtype=np.float32)[0, 0].T)  # [HD, S]
    sinT = np.ascontiguousarray(np.asarray(sin, dtype=np.float32)[0, 0].T)
    sinTs = sinT.copy()
    sinTs[0:HD // 2] = -sinT[0:HD // 2]

    mask01 = np.zeros((128, 640), np.float32)
    tri = (np.arange(128)[:, None] <= np.arange(128)[None, :])
    mask01[:, 512:640] = tri.astype(np.float32)
    ident = np.eye(128, dtype=np.float32)
    ones = np.ones((128, 1), np.float32)

    in_maps = []
    for d in range(8):
        in_maps.append({
            "hsT": hsT,
            "wqT": np.ascontiguousarray(wqsT[:, d * QD:(d + 1) * QD]),
            "wkT": np.ascontiguousarray(wkT[:, d * HD:(d + 1) * HD]),
            "wvT": np.ascontiguousarray(wvT[:, d * HD:(d + 1) * HD]),
            "wdT": np.ascontiguousarray(wdT[d * QD:(d + 1) * QD, :]),
            "cosT": cosT,
            "sinTs": sinTs,
            "mask01": mask01,
            "ident": ident,
            "ones": ones,
        })
    return in_maps


def _gather(results):
    outT = np.zeros((H, NT), np.float64)
    for d in range(8):
        outT += results[d]["outT"]
    out = outT.T.astype(np.float32).reshape(B, S, H)
    k = np.stack([results[d]["kT"].reshape(HD, B, S).transpose(1, 2, 0)
                  for d in range(8)], axis=1)             # [B, NKV, S, HD]
    v = np.stack([results[d]["vT"].reshape(HD, B, S).transpose(1, 2, 0)
                  for d in range(8)], axis=1)
    return out, np.ascontiguousarray(k), np.ascontiguousarray(v)


def _run(in_maps, **kwargs):
    if "nc" not in _CACHE:
        _CACHE["nc"] = _build_nc()
    return run_bass_kernel_spmd(_CACHE["nc"], in_maps, list(range(8)), **kwargs)


def kernel(hidden_states, cos, sin, wq, wk, wv, wd):
    in_maps = _host_inputs(hidden_states, cos, sin, wq, wk, wv, wd)
    res = _run(in_maps)
    return _gather(res.results)


# revision 6
# speedup vs baseline: 1.1530x; 1.0765x over previous
"""GQA attention block (B=2,S=1024,H=4096, 32 q-heads / 8 kv-heads, RoPE, causal)
on 8 trn2 NeuronCores.

Sharding: tensor-parallel over heads. Core d owns kv-head d and q-heads
4d..4d+3. Each core computes its 4 attention heads end-to-end plus the partial
dense projection out_part.T = wd[:, cols_d] @ ctx_d.T; the host sums the 8
fp32 partials (gather is host-side anyway, so no device collective is needed).

All matmuls run in float32r (full PE rate, ~1.4e-4 rel err vs fp32).
Layout on chip is feature-major ("transposed"): activations hsT [H, n],
Q/K as [hd, n] so the PE contracts over partitions everywhere; host does all
the cheap numpy transposes.
"""
import math
import numpy as np

import concourse.bass as bass
import concourse.tile as tile
from concourse import bacc, mybir
from concourse.bass_utils import run_bass_kernel_spmd

B, S, H = 2, 1024, 4096
NH, NKV, HD = 32, 8, 128
GROUP = NH // NKV          # q-heads per kv-head = per-core q-heads
NT = B * S                 # 2048 tokens
QD = GROUP * HD            # 512 per-core q features
NCH = NT // 512            # 4 token chunks of 512
HCH = H // 128             # 32 contraction chunks

f32 = mybir.dt.float32
f32r = mybir.dt.float32r
AF = mybir.ActivationFunctionType

_CACHE = {}


def _build_nc():
    nc = bacc.Bacc("TRN2", target_bir_lowering=False, debug=False, num_devices=8)
    io = {}
    io["hsT"] = nc.dram_tensor("hsT", [H, NT], f32r, kind="ExternalInput").ap()
    io["wqT"] = nc.dram_tensor("wqT", [H, QD], f32r, kind="ExternalInput").ap()
    io["wkT"] = nc.dram_tensor("wkT", [H, HD], f32r, kind="ExternalInput").ap()
    io["wvT"] = nc.dram_tensor("wvT", [H, HD], f32r, kind="ExternalInput").ap()
    io["wdT"] = nc.dram_tensor("wdT", [QD, H], f32r, kind="ExternalInput").ap()
    io["cosT"] = nc.dram_tensor("cosT", [HD, S], f32, kind="ExternalInput").ap()
    io["sinTs"] = nc.dram_tensor("sinTs", [HD, S], f32, kind="ExternalInput").ap()
    io["mask01"] = nc.dram_tensor("mask01", [128, 640], f32, kind="ExternalInput").ap()
    io["ident"] = nc.dram_tensor("ident", [128, 128], f32, kind="ExternalInput").ap()
    io["ones"] = nc.dram_tensor("ones", [128, 128], f32r, kind="ExternalInput").ap()
    io["outT"] = nc.dram_tensor("outT", [H, NT], f32, kind="ExternalOutput").ap()
    io["kT"] = nc.dram_tensor("kT", [HD, NT], f32, kind="ExternalOutput").ap()
    io["vT"] = nc.dram_tensor("vT", [HD, NT], f32, kind="ExternalOutput").ap()

    with tile.TileContext(nc) as tc:
        _emit(nc, tc, io)
    nc.compile()
    return nc


def _emit(nc, tc, io):
    from contextlib import ExitStack

    ctx = ExitStack()
    with ctx:
        const_p = ctx.enter_context(tc.tile_pool(name="const", bufs=1))
        persist_p = ctx.enter_context(tc.tile_pool(name="persist", bufs=1))
        hs_p = ctx.enter_context(tc.tile_pool(name="hs", bufs=2))
        wq_p = ctx.enter_context(tc.tile_pool(name="wq", bufs=2))
        wkv_p = ctx.enter_context(tc.tile_pool(name="wkv", bufs=2))
        rope_p = ctx.enter_context(tc.tile_pool(name="rope", bufs=2))
        exp_p = ctx.enter_context(tc.tile_pool(name="exp", bufs=2))
        nrm_p = ctx.enter_context(tc.tile_pool(name="nrm", bufs=2))
        stg_p = ctx.enter_context(tc.tile_pool(name="stg", bufs=6))
        wd_p = ctx.enter_context(tc.tile_pool(name="wd", bufs=4))
        osb_p = ctx.enter_context(tc.tile_pool(name="osb", bufs=2))

        # ---- constants / small tables ----
        cos_sb = const_p.tile([HD, S], f32, tag="cos", name="cos")
        nc.sync.dma_start(cos_sb[:], io["cosT"])
        sin_sb = const_p.tile([HD, S], f32, tag="sin", name="sin")
        nc.sync.dma_start(sin_sb[:], io["sinTs"])
        mask_sb = const_p.tile([128, 640], f32, tag="mask", name="mask")
        nc.sync.dma_start(mask_sb[:], io["mask01"])
        ident_sb = const_p.tile([128, 128], f32, tag="ident", name="ident")
        nc.sync.dma_start(ident_sb[:], io["ident"])
        ones_sb = const_p.tile([128, 128], f32r, tag="ones", name="ones")
        nc.sync.dma_start(ones_sb[:], io["ones"])

        # ---- persistent activations ----
        qt = [persist_p.tile([128, NT], f32r, tag=f"qt{j}", name=f"qt{j}") for j in range(GROUP)]
        ktr = persist_p.tile([128, NT], f32r, tag="ktr", name="ktr")
        vnat = [persist_p.tile([128, 128], f32r, tag=f"vn{i}", name=f"vn{i}") for i in range(NT // 128)]
        ctxT = [persist_p.tile([128, NT], f32r, tag=f"cx{j}", name=f"cx{j}") for j in range(GROUP)]

        HB = 4          # h-chunks per batched DMA
        hsT_r = io["hsT"].rearrange("(k p) n -> p k n", p=128)   # [128, 32, 2048]
        wqT_r = io["wqT"].rearrange("(k p) d -> p k d", p=128)   # [128, 32, 512]
        wkT_r = io["wkT"].rearrange("(k p) d -> p k d", p=128)   # [128, 32, 128]
        wvT_r = io["wvT"].rearrange("(k p) d -> p k d", p=128)

        # ================= stage A: QKV projections + RoPE =================
        ps_a_ctx = ExitStack()
        ps_a = ps_a_ctx.enter_context(tc.tile_pool(name="ps_a", bufs=1, space="PSUM"))
        for c in range(NCH):
            n0 = c * 512
            qps = [ps_a.tile([128, 512], f32, tag=f"qps{j}", name=f"qps{j}") for j in range(GROUP)]
            kps = ps_a.tile([128, 512], f32, tag="kps", name="kps")
            vps = ps_a.tile([128, 512], f32, tag="vps", name="vps")
            for hb in range(HCH // HB):
                h0 = hb * HB
                hs_t = hs_p.tile([128, HB, 512], f32r, tag="hs", name="hs")
                nc.sync.dma_start(hs_t[:], hsT_r[:, h0:h0 + HB, n0:n0 + 512])
                wq_t = wq_p.tile([128, HB, QD], f32r, tag="wq", name="wq")
                nc.sync.dma_start(wq_t[:], wqT_r[:, h0:h0 + HB, :])
                wk_t = wkv_p.tile([128, HB, HD], f32r, tag="wk", name="wk")
                nc.sync.dma_start(wk_t[:], wkT_r[:, h0:h0 + HB, :])
                wv_t = wkv_p.tile([128, HB, HD], f32r, tag="wv", name="wv")
                nc.sync.dma_start(wv_t[:], wvT_r[:, h0:h0 + HB, :])
                for i in range(HB):
                    h = h0 + i
                    st = (h == 0)
                    sp = (h == HCH - 1)
                    for j in range(GROUP):
                        nc.tensor.matmul(qps[j][:], wq_t[:, i, j * 128:(j + 1) * 128],
                                         hs_t[:, i, :], start=st, stop=sp)
                    nc.tensor.matmul(kps[:], wk_t[:, i, :], hs_t[:, i, :], start=st, stop=sp)
                    nc.tensor.matmul(vps[:], wv_t[:, i, :], hs_t[:, i, :], start=st, stop=sp)

            off = (c % (S // 512)) * 512  # position offset within batch
            cos_sl = cos_sb[:, off:off + 512]
            sin_sl = sin_sb[:, off:off + 512]
            # stage all 6 psums to SBUF fast (split ACT/DVE) so banks free early
            qsb = [stg_p.tile([128, 512], f32, tag="stg", name="stg") for _ in range(GROUP)]
            ksb = stg_p.tile([128, 512], f32, tag="stg", name="stg")
            vt_out = rope_p.tile([128, 512], f32, tag="vstage", name="vstage")
            nc.scalar.copy(qsb[0][:], qps[0][:])
            nc.vector.tensor_copy(qsb[1][:], qps[1][:])
            nc.scalar.copy(qsb[2][:], qps[2][:])
            nc.vector.tensor_copy(qsb[3][:], qps[3][:])
            nc.scalar.copy(ksb[:], kps[:])
            nc.vector.tensor_copy(vt_out[:], vps[:])
            nc.sync.dma_start(io["vT"][:, n0:n0 + 512], vt_out[:])
            # Q RoPE -> qt[j] (f32r)
            for j in range(GROUP):
                rq = rope_p.tile([128, 512], f32, tag="rq", name="rq")
                nc.vector.tensor_copy(rq[0:64, :], qsb[j][64:128, :])
                nc.vector.tensor_copy(rq[64:128, :], qsb[j][0:64, :])
                nc.vector.tensor_mul(rq[:], rq[:], sin_sl)
                qc_t = rope_p.tile([128, 512], f32, tag="qcos", name="qcos")
                nc.vector.tensor_mul(qc_t[:], qsb[j][:], cos_sl)
                nc.vector.tensor_add(qt[j][:, n0:n0 + 512], qc_t[:], rq[:])
            # K RoPE -> kt_out (f32, staged for DMA) + ktr (f32r)
            rk = rope_p.tile([128, 512], f32, tag="rk", name="rk")
            nc.vector.tensor_copy(rk[0:64, :], ksb[64:128, :])
            nc.vector.tensor_copy(rk[64:128, :], ksb[0:64, :])
            nc.vector.tensor_mul(rk[:], rk[:], sin_sl)
            kt_out = rope_p.tile([128, 512], f32, tag="kcos", name="kcos")
            nc.vector.tensor_mul(kt_out[:], ksb[:], cos_sl)
            nc.vector.tensor_add(kt_out[:], kt_out[:], rk[:])
            nc.scalar.copy(ktr[:, n0:n0 + 512], kt_out[:])
            nc.sync.dma_start(io["kT"][:, n0:n0 + 512], kt_out[:])
            # V transpose -> vnat (f32r), 4 tiles of 128
            for i in range(4):
                g = c * 4 + i
                tp = ps_a.tile([128, 128], f32, tag="tp", name="tp")
                nc.tensor.transpose(tp[:], vt_out[:, i * 128:(i + 1) * 128], ident_sb[:])
                nc.vector.tensor_copy(vnat[g][:], tp[:])

        ps_a_ctx.close()
        # ================= stage B: attention per (batch, q-head) =================
        ps_b_ctx = ExitStack()
        ps_b = ps_b_ctx.enter_context(tc.tile_pool(name="ps_b", bufs=1, space="PSUM"))
        for b in range(B):
            base = b * S
            for j in range(GROUP):
                cps = ps_b.tile([128, S], f32, tag="cps", name="cps")    # ctxT_un [hd, q]
                dps = ps_b.tile([128, S], f32, tag="dps", name="dps")    # denom bcast [*, q]
                for t in range(S // 128):                    # key chunks
                    qcd = t // 4                             # q-chunk holding the diagonal
                    sps = ps_b.tile([128, S], f32, tag="sps", name="sps", bufs=2)
                    et = exp_p.tile([128, S], f32r, tag="et", name="et")
                    for qc in range(S // 512):
                        if qc < qcd:
                            continue                          # fully masked
                        q0 = qc * 512
                        nc.tensor.matmul(sps[:, q0:q0 + 512],
                                         ktr[:, base + t * 128:base + (t + 1) * 128],
                                         qt[j][:, base + q0:base + q0 + 512])
                        nc.scalar.activation(et[:, q0:q0 + 512], sps[:, q0:q0 + 512], AF.Exp)
                    # zero the masked region: columns [qcd*512 .. t*128+128)
                    o = t * 128 - qcd * 512
                    nc.vector.tensor_mul(et[:, qcd * 512:t * 128 + 128],
                                         et[:, qcd * 512:t * 128 + 128].bitcast(f32),
                                         mask_sb[:, 512 - o:640])
                    for qc in range(S // 512):
                        if qc < qcd:
                            continue
                        q0 = qc * 512
                        st = (t == 0)
                        sp = (t == (qc * 4 + 3))
                        nc.tensor.matmul(cps[:, q0:q0 + 512], vnat[b * 8 + t][:],
                                         et[:, q0:q0 + 512], start=st, stop=sp)
                        nc.tensor.matmul(dps[:, q0:q0 + 512], ones_sb[:],
                                         et[:, q0:q0 + 512], start=st, stop=sp)
                rec = nrm_p.tile([128, S], f32, tag="rec", name="rec")
                scr = nrm_p.tile([128, S], f32, tag="scr", name="scr")
                nc.vector.reciprocal_approx_accurate(rec[:], dps[:], scr[:])
                nc.vector.tensor_mul(ctxT[j][:, base:base + S], cps[:], rec[:])

        ps_b_ctx.close()
        # ================= stage C: dense partial out.T = wd_d @ ctx_d.T =============
        ps_c = ctx.enter_context(tc.tile_pool(name="ps_c", bufs=1, space="PSUM"))
        wdT_r = io["wdT"].rearrange("(k p) n -> p k n", p=128)   # [128, 4, 4096]
        for cp in range(2):                                  # batch halves (n chunks 2cp, 2cp+1)
            for ho in range(H // 128):
                ops = [ps_c.tile([128, 512], f32, tag=f"ops{i}", name=f"ops{i}", bufs=2)
                       for i in range(2)]
                wd_t = wd_p.tile([128, GROUP, 128], f32r, tag="wd", name="wd")
                nc.sync.dma_start(wd_t[:], wdT_r[:, :, ho * 128:(ho + 1) * 128])
                for ci in range(GROUP):
                    for i in range(2):
                        n0 = (2 * cp + i) * 512
                        nc.tensor.matmul(ops[i][:], wd_t[:, ci, :], ctxT[ci][:, n0:n0 + 512],
                                         start=(ci == 0), stop=(ci == GROUP - 1))
                for i in range(2):
                    n0 = (2 * cp + i) * 512
                    ot = osb_p.tile([128, 512], f32, tag="ot", name="ot")
                    if i == 0:
                        nc.scalar.copy(ot[:], ops[i][:])
                    else:
                        nc.vector.tensor_copy(ot[:], ops[i][:])
                    nc.sync.dma_start(io["outT"][ho * 128:(ho + 1) * 128, n0:n0 + 512], ot[:])


def _host_inputs(hidden_states, cos, sin, wq, wk, wv, wd):
    """Build the 8 per-core input maps (all numpy, fp32)."""
    hs = np.asarray(hidden_states, dtype=np.float32).reshape(NT, H)
    hsT = np.ascontiguousarray(hs.T)
    scale = np.float32(1.0 / math.sqrt(HD))
    wq = np.asarray(wq, dtype=np.float32)
    wk = np.asarray(wk, dtype=np.float32)
    wv = np.asarray(wv, dtype=np.float32)
    wd = np.asarray(wd, dtype=np.float32)
    wqsT = np.ascontiguousarray((wq * scale).T)        # [H, NH*HD]
    wkT = np.ascontiguousarray(wk.T)                   # [H, NKV*HD]
    wvT = np.ascontiguousarray(wv.T)
    wdT = np.ascontiguousarray(wd.T)                   # [H_in(ctx), H_out]

    cosT = np.ascontiguousarray(np.asarray(cos, dtype=np.float32)[0, 0].T)  # [HD, S]
    sinT = np.ascontiguousarray(np.asarray(sin, dtype=np.float32)[0, 0].T)
    sinTs = sinT.copy()
    sinTs[0:HD // 2] = -sinT[0:HD // 2]

    mask01 = np.zeros((128, 640), np.float32)
    tri = (np.arange(128)[:, None] <= np.arange(128)[None, :])
    mask01[:, 512:640] = tri.astype(np.float32)
    ident = np.eye(128, dtype=np.float32)
    ones = np.ones((128, 128), np.float32)

    in_maps = []
    for d in range(8):
        in_maps.append({
            "hsT": hsT,
            "wqT": np.ascontiguousarray(wqsT[:, d * QD:(d + 1) * QD]),
            "wkT": np.ascontiguousarray(wkT[:, d * HD:(d + 1) * HD]),
            "wvT": np.ascontiguousarray(wvT[:, d * HD:(d + 1) * HD]),
            "wdT": np.ascontiguousarray(wdT[d * QD:(d + 1) * QD, :]),
            "cosT": cosT,
            "sinTs": sinTs,
            "mask01": mask01,
            "ident": ident,
            "ones": ones,
        })
    return in_maps


def _gather(results):
    outT = np.zeros((H, NT), np.float64)
    for d in range(8):
        outT += results[d]["outT"]
    out = outT.T.astype(np.float32).reshape(B, S, H)
    k = np.stack([results[d]["kT"].reshape(HD, B, S).transpose(1, 2, 0)
                  for d in range(8)], axis=1)             # [B, NKV, S, HD]
    v = np.stack([results[d]["vT"].reshape(HD, B, S).transpose(1, 2, 0)
                  for d in range(8)], axis=1)
    return out, np.ascontiguousarray(k), np.ascontiguousarray(v)


def _run(in_maps, **kwargs):
    if "nc" not in _CACHE:
        _CACHE["nc"] = _build_nc()
    return run_bass_kernel_spmd(_CACHE["nc"], in_maps, list(range(8)), **kwargs)


def kernel(hidden_states, cos, sin, wq, wk, wv, wd):
    in_maps = _host_inputs(hidden_states, cos, sin, wq, wk, wv, wd)
    res = _run(in_maps)
    return _gather(res.results)


# revision 8
# speedup vs baseline: 1.2742x; 1.1051x over previous
"""GQA attention block (B=2,S=1024,H=4096, 32 q-heads / 8 kv-heads, RoPE, causal)
on 8 trn2 NeuronCores.

Sharding: tensor-parallel over heads. Core d owns kv-head d and q-heads
4d..4d+3. Each core computes its 4 attention heads end-to-end plus the partial
dense projection out_part.T = wd[:, cols_d] @ ctx_d.T; the host sums the 8
fp32 partials (gather is host-side anyway, so no device collective is needed).

All matmuls run in float32r (full PE rate, ~1.4e-4 rel err vs fp32).
Layout on chip is feature-major ("transposed"): activations hsT [H, n],
Q/K as [hd, n] so the PE contracts over partitions everywhere; host does all
the cheap numpy transposes.
"""
import math
import numpy as np

import concourse.bass as bass
import concourse.tile as tile
from concourse import bacc, mybir
from concourse.bass_utils import run_bass_kernel_spmd

B, S, H = 2, 1024, 4096
NH, NKV, HD = 32, 8, 128
GROUP = NH // NKV          # q-heads per kv-head = per-core q-heads
NT = B * S                 # 2048 tokens
QD = GROUP * HD            # 512 per-core q features
NCH = NT // 512            # 4 token chunks of 512
HCH = H // 128             # 32 contraction chunks

f32 = mybir.dt.float32
f32r = mybir.dt.float32r
AF = mybir.ActivationFunctionType

_CACHE = {}


def _build_nc():
    nc = bacc.Bacc("TRN2", target_bir_lowering=False, debug=False, num_devices=8)
    io = {}
    io["hsB"] = nc.dram_tensor("hsB", [NCH, 8, 128, 4, 512], f32r, kind="ExternalInput").ap()
    io["wqB"] = nc.dram_tensor("wqB", [8, 128, 4, QD], f32r, kind="ExternalInput").ap()
    io["wkB"] = nc.dram_tensor("wkB", [8, 128, 4, HD], f32r, kind="ExternalInput").ap()
    io["wvB"] = nc.dram_tensor("wvB", [8, 128, 4, HD], f32r, kind="ExternalInput").ap()
    io["wdB"] = nc.dram_tensor("wdB", [32, 128, 4, 128], f32r, kind="ExternalInput").ap()
    io["cosT"] = nc.dram_tensor("cosT", [HD, S], f32, kind="ExternalInput").ap()
    io["sinTs"] = nc.dram_tensor("sinTs", [HD, S], f32, kind="ExternalInput").ap()
    io["mask01"] = nc.dram_tensor("mask01", [128, 640], f32, kind="ExternalInput").ap()
    io["ident"] = nc.dram_tensor("ident", [128, 128], f32, kind="ExternalInput").ap()
    io["ones"] = nc.dram_tensor("ones", [128, 128], f32r, kind="ExternalInput").ap()
    io["outT"] = nc.dram_tensor("outT", [H, NT], f32, kind="ExternalOutput").ap()
    io["kT"] = nc.dram_tensor("kT", [HD, NT], f32, kind="ExternalOutput").ap()
    io["vT"] = nc.dram_tensor("vT", [HD, NT], f32, kind="ExternalOutput").ap()

    with tile.TileContext(nc) as tc:
        _emit(nc, tc, io)
    nc.compile()
    return nc


def _emit(nc, tc, io):
    from contextlib import ExitStack

    ctx = ExitStack()
    with ctx:
        const_p = ctx.enter_context(tc.tile_pool(name="const", bufs=1))
        persist_p = ctx.enter_context(tc.tile_pool(name="persist", bufs=1))
        hs_p = ctx.enter_context(tc.tile_pool(name="hs", bufs=2))
        wq_p = ctx.enter_context(tc.tile_pool(name="wq", bufs=2))
        wkv_p = ctx.enter_context(tc.tile_pool(name="wkv", bufs=2))
        rope_p = ctx.enter_context(tc.tile_pool(name="rope", bufs=2))
        exp_p = ctx.enter_context(tc.tile_pool(name="exp", bufs=2))
        nrm_p = ctx.enter_context(tc.tile_pool(name="nrm", bufs=1))
        stg_p = ctx.enter_context(tc.tile_pool(name="stg", bufs=6))
        wd_p = ctx.enter_context(tc.tile_pool(name="wd", bufs=6))
        osb_p = ctx.enter_context(tc.tile_pool(name="osb", bufs=2))

        # ---- constants / small tables ----
        cos_sb = const_p.tile([HD, S], f32, tag="cos", name="cos")
        nc.sync.dma_start(cos_sb[:], io["cosT"])
        sin_sb = const_p.tile([HD, S], f32, tag="sin", name="sin")
        nc.sync.dma_start(sin_sb[:], io["sinTs"])
        mask_sb = const_p.tile([128, 640], f32, tag="mask", name="mask")
        nc.sync.dma_start(mask_sb[:], io["mask01"])
        ident_sb = const_p.tile([128, 128], f32, tag="ident", name="ident")
        nc.sync.dma_start(ident_sb[:], io["ident"])
        ones_sb = const_p.tile([128, 128], f32r, tag="ones", name="ones")
        nc.sync.dma_start(ones_sb[:], io["ones"])

        # ---- persistent activations ----
        qt = [persist_p.tile([128, NT], f32r, tag=f"qt{j}", name=f"qt{j}") for j in range(GROUP)]
        ktr = persist_p.tile([128, NT], f32r, tag="ktr", name="ktr")
        vnat = [persist_p.tile([128, 128], f32r, tag=f"vn{i}", name=f"vn{i}") for i in range(NT // 128)]
        ctxT = [persist_p.tile([128, NT], f32r, tag=f"cx{j}", name=f"cx{j}") for j in range(GROUP)]

        HB = 4          # h-chunks per batched DMA

        # ================= stage A: QKV projections + RoPE =================
        ps_a_ctx = ExitStack()
        ps_a = ps_a_ctx.enter_context(tc.tile_pool(name="ps_a", bufs=1, space="PSUM"))
        for c in range(NCH):
            n0 = c * 512
            qps = [ps_a.tile([128, 512], f32, tag=f"qps{j}", name=f"qps{j}") for j in range(GROUP)]
            kps = ps_a.tile([128, 512], f32, tag="kps", name="kps")
            vps = ps_a.tile([128, 512], f32, tag="vps", name="vps")
            for hb in range(HCH // HB):
                h0 = hb * HB
                hs_t = hs_p.tile([128, HB, 512], f32r, tag="hs", name="hs")
                nc.sync.dma_start(hs_t[:], io["hsB"][c, hb])
                wq_t = wq_p.tile([128, HB, QD], f32r, tag="wq", name="wq")
                nc.sync.dma_start(wq_t[:], io["wqB"][hb])
                wk_t = wkv_p.tile([128, HB, HD], f32r, tag="wk", name="wk")
                nc.sync.dma_start(wk_t[:], io["wkB"][hb])
                wv_t = wkv_p.tile([128, HB, HD], f32r, tag="wv", name="wv")
                nc.sync.dma_start(wv_t[:], io["wvB"][hb])
                for i in range(HB):
                    h = h0 + i
                    st = (h == 0)
                    sp = (h == HCH - 1)
                    for j in range(GROUP):
                        nc.tensor.matmul(qps[j][:], wq_t[:, i, j * 128:(j + 1) * 128],
                                         hs_t[:, i, :], start=st, stop=sp)
                    nc.tensor.matmul(kps[:], wk_t[:, i, :], hs_t[:, i, :], start=st, stop=sp)
                    nc.tensor.matmul(vps[:], wv_t[:, i, :], hs_t[:, i, :], start=st, stop=sp)

            off = (c % (S // 512)) * 512  # position offset within batch
            cos_sl = cos_sb[:, off:off + 512]
            sin_sl = sin_sb[:, off:off + 512]
            # stage all 6 psums to SBUF fast (split ACT/DVE) so banks free early
            qsb = [stg_p.tile([128, 512], f32, tag="stg", name="stg") for _ in range(GROUP)]
            ksb = stg_p.tile([128, 512], f32, tag="stg", name="stg")
            vt_out = rope_p.tile([128, 512], f32, tag="vstage", name="vstage")
            nc.scalar.copy(qsb[0][:], qps[0][:])
            nc.vector.tensor_copy(qsb[1][:], qps[1][:])
            nc.scalar.copy(qsb[2][:], qps[2][:])
            nc.vector.tensor_copy(qsb[3][:], qps[3][:])
            nc.scalar.copy(ksb[:], kps[:])
            nc.vector.tensor_copy(vt_out[:], vps[:])
            nc.sync.dma_start(io["vT"][:, n0:n0 + 512], vt_out[:])
            # Q RoPE -> qt[j] (f32r)
            for j in range(GROUP):
                rq = rope_p.tile([128, 512], f32, tag="rq", name="rq")
                nc.vector.tensor_copy(rq[0:64, :], qsb[j][64:128, :])
                nc.vector.tensor_copy(rq[64:128, :], qsb[j][0:64, :])
                nc.vector.tensor_mul(rq[:], rq[:], sin_sl)
                qc_t = rope_p.tile([128, 512], f32, tag="qcos", name="qcos")
                nc.vector.tensor_mul(qc_t[:], qsb[j][:], cos_sl)
                nc.vector.tensor_add(qt[j][:, n0:n0 + 512], qc_t[:], rq[:])
            # K RoPE -> kt_out (f32, staged for DMA) + ktr (f32r)
            rk = rope_p.tile([128, 512], f32, tag="rk", name="rk")
            nc.vector.tensor_copy(rk[0:64, :], ksb[64:128, :])
            nc.vector.tensor_copy(rk[64:128, :], ksb[0:64, :])
            nc.vector.tensor_mul(rk[:], rk[:], sin_sl)
            kt_out = rope_p.tile([128, 512], f32, tag="kcos", name="kcos")
            nc.vector.tensor_mul(kt_out[:], ksb[:], cos_sl)
            nc.vector.tensor_add(kt_out[:], kt_out[:], rk[:])
            nc.scalar.copy(ktr[:, n0:n0 + 512], kt_out[:])
            nc.sync.dma_start(io["kT"][:, n0:n0 + 512], kt_out[:])
            # V transpose -> vnat (f32r), 4 tiles of 128
            for i in range(4):
                g = c * 4 + i
                tp = ps_a.tile([128, 128], f32, tag="tp", name="tp")
                nc.tensor.transpose(tp[:], vt_out[:, i * 128:(i + 1) * 128], ident_sb[:])
                nc.vector.tensor_copy(vnat[g][:], tp[:])

        ps_a_ctx.close()
        # ================= stage B: attention per (batch, q-head) =================
        ps_b_ctx = ExitStack()
        ps_b = ps_b_ctx.enter_context(tc.tile_pool(name="ps_b", bufs=1, space="PSUM"))
        for b in range(B):
            base = b * S
            for j in range(GROUP):
                cps = ps_b.tile([128, S], f32, tag="cps", name="cps")    # ctxT_un [hd, q]
                dps = ps_b.tile([128, S], f32, tag="dps", name="dps")    # denom bcast [*, q]
                for t in range(S // 128):                    # key chunks
                    qcd = t // 4                             # q-chunk holding the diagonal
                    sps = ps_b.tile([128, S], f32, tag="sps", name="sps", bufs=2)
                    et = exp_p.tile([128, S], f32r, tag="et", name="et")
                    for qc in range(S // 512):
                        if qc < qcd:
                            continue                          # fully masked
                        q0 = qc * 512
                        nc.tensor.matmul(sps[:, q0:q0 + 512],
                                         ktr[:, base + t * 128:base + (t + 1) * 128],
                                         qt[j][:, base + q0:base + q0 + 512])
                        nc.scalar.activation(et[:, q0:q0 + 512], sps[:, q0:q0 + 512], AF.Exp)
                    # zero the masked region: columns [qcd*512 .. t*128+128)
                    o = t * 128 - qcd * 512
                    nc.vector.tensor_mul(et[:, qcd * 512:t * 128 + 128],
                                         et[:, qcd * 512:t * 128 + 128].bitcast(f32),
                                         mask_sb[:, 512 - o:640])
                    for qc in range(S // 512):
                        if qc < qcd:
                            continue
                        q0 = qc * 512
                        st = (t == 0)
                        sp = (t == (qc * 4 + 3))
                        nc.tensor.matmul(cps[:, q0:q0 + 512], vnat[b * 8 + t][:],
                                         et[:, q0:q0 + 512], start=st, stop=sp)
                        nc.tensor.matmul(dps[:, q0:q0 + 512], ones_sb[:],
                                         et[:, q0:q0 + 512], start=st, stop=sp)
                rec = nrm_p.tile([128, S], f32, tag="rec", name="rec")
                scr = nrm_p.tile([128, S], f32, tag="scr", name="scr")
                nc.vector.reciprocal_approx_accurate(rec[:], dps[:], scr[:])
                nc.vector.tensor_mul(ctxT[j][:, base:base + S], cps[:], rec[:])

        ps_b_ctx.close()
        # ================= stage C: dense partial out.T = wd_d @ ctx_d.T =============
        ps_c = ctx.enter_context(tc.tile_pool(name="ps_c", bufs=1, space="PSUM"))
        for cp in range(2):                                  # batch halves (n chunks 2cp, 2cp+1)
            for ho in range(H // 128):
                ops = [ps_c.tile([128, 512], f32, tag=f"ops{i}", name=f"ops{i}", bufs=2)
                       for i in range(2)]
                wd_t = wd_p.tile([128, GROUP, 128], f32r, tag="wd", name="wd")
                nc.sync.dma_start(wd_t[:], io["wdB"][ho])
                for ci in range(GROUP):
                    for i in range(2):
                        n0 = (2 * cp + i) * 512
                        nc.tensor.matmul(ops[i][:], wd_t[:, ci, :], ctxT[ci][:, n0:n0 + 512],
                                         start=(ci == 0), stop=(ci == GROUP - 1))
                for i in range(2):
                    n0 = (2 * cp + i) * 512
                    ot = osb_p.tile([128, 512], f32, tag="ot", name="ot")
                    if i == 0:
                        nc.scalar.copy(ot[:], ops[i][:])
                    else:
                        nc.vector.tensor_copy(ot[:], ops[i][:])
                    nc.sync.dma_start(io["outT"][ho * 128:(ho + 1) * 128, n0:n0 + 512], ot[:])


def _host_inputs(hidden_states, cos, sin, wq, wk, wv, wd):
    """Build the 8 per-core input maps (all numpy, fp32)."""
    hs = np.asarray(hidden_states, dtype=np.float32).reshape(NT, H)
    hsT = np.ascontiguousarray(hs.T)
    scale = np.float32(1.0 / math.sqrt(HD))
    wq = np.asarray(wq, dtype=np.float32)
    wk = np.asarray(wk, dtype=np.float32)
    wv = np.asarray(wv, dtype=np.float32)
    wd = np.asarray(wd, dtype=np.float32)
    wqsT = (wq * scale).T                              # [H, NH*HD]
    wkT = wk.T                                         # [H, NKV*HD]
    wvT = wv.T
    wdT = wd.T                                         # [H_in(ctx), H_out]
    # blocked DMA layouts: every on-chip DMA reads one contiguous block
    hsB = np.ascontiguousarray(
        hsT.reshape(8, 4, 128, NCH, 512).transpose(3, 0, 2, 1, 4))

    cosT = np.ascontiguousarray(np.asarray(cos, dtype=np.float32)[0, 0].T)  # [HD, S]
    sinT = np.ascontiguousarray(np.asarray(sin, dtype=np.float32)[0, 0].T)
    sinTs = sinT.copy()
    sinTs[0:HD // 2] = -sinT[0:HD // 2]

    mask01 = np.zeros((128, 640), np.float32)
    tri = (np.arange(128)[:, None] <= np.arange(128)[None, :])
    mask01[:, 512:640] = tri.astype(np.float32)
    ident = np.eye(128, dtype=np.float32)
    ones = np.ones((128, 128), np.float32)

    in_maps = []
    for d in range(8):
        wq_d = wqsT[:, d * QD:(d + 1) * QD]            # [H, 512]
        wk_d = wkT[:, d * HD:(d + 1) * HD]
        wv_d = wvT[:, d * HD:(d + 1) * HD]
        wd_d = wdT[d * QD:(d + 1) * QD, :]             # [512, H]
        in_maps.append({
            "hsB": hsB,
            "wqB": np.ascontiguousarray(wq_d.reshape(8, 4, 128, QD).transpose(0, 2, 1, 3)),
            "wkB": np.ascontiguousarray(wk_d.reshape(8, 4, 128, HD).transpose(0, 2, 1, 3)),
            "wvB": np.ascontiguousarray(wv_d.reshape(8, 4, 128, HD).transpose(0, 2, 1, 3)),
            "wdB": np.ascontiguousarray(wd_d.reshape(4, 128, 32, 128).transpose(2, 1, 0, 3)),
            "cosT": cosT,
            "sinTs": sinTs,
            "mask01": mask01,
            "ident": ident,
            "ones": ones,
        })
    return in_maps


def _gather(results):
    outT = np.zeros((H, NT), np.float64)
    for d in range(8):
        outT += results[d]["outT"]
    out = outT.T.astype(np.float32).reshape(B, S, H)
    k = np.stack([results[d]["kT"].reshape(HD, B, S).transpose(1, 2, 0)
                  for d in range(8)], axis=1)             # [B, NKV, S, HD]
    v = np.stack([results[d]["vT"].reshape(HD, B, S).transpose(1, 2, 0)
                  for d in range(8)], axis=1)
    return out, np.ascontiguousarray(k), np.ascontiguousarray(v)


def _run(in_maps, **kwargs):
    if "nc" not in _CACHE:
        _CACHE["nc"] = _build_nc()
    return run_bass_kernel_spmd(_CACHE["nc"], in_maps, list(range(8)), **kwargs)


def kernel(hidden_states, cos, sin, wq, wk, wv, wd):
    in_maps = _host_inputs(hidden_states, cos, sin, wq, wk, wv, wd)
    res = _run(in_maps)
    return _gather(res.results)


# revision 9
# speedup vs baseline: 1.4116x; 1.1079x over previous
"""GQA attention block (B=2,S=1024,H=4096, 32 q-heads / 8 kv-heads, RoPE, causal)
on 8 trn2 NeuronCores.

Sharding: tensor-parallel over heads. Core d owns kv-head d and q-heads
4d..4d+3. Each core computes its 4 attention heads end-to-end plus the partial
dense projection out_part.T = wd[:, cols_d] @ ctx_d.T; the host sums the 8
fp32 partials (gather is host-side anyway, so no device collective is needed).

All matmuls run in float32r (full PE rate, ~1.4e-4 rel err vs fp32).
Layout on chip is feature-major ("transposed"): activations hsT [H, n],
Q/K as [hd, n] so the PE contracts over partitions everywhere; host does all
the cheap numpy transposes.
"""
import math
import numpy as np

import concourse.bass as bass
import concourse.tile as tile
from concourse import bacc, mybir
from concourse.bass_utils import run_bass_kernel_spmd

B, S, H = 2, 1024, 4096
NH, NKV, HD = 32, 8, 128
GROUP = NH // NKV          # q-heads per kv-head = per-core q-heads
NT = B * S                 # 2048 tokens
QD = GROUP * HD            # 512 per-core q features
NCH = NT // 512            # 4 token chunks of 512
HCH = H // 128             # 32 contraction chunks

f32 = mybir.dt.float32
f32r = mybir.dt.float32r
AF = mybir.ActivationFunctionType

_CACHE = {}


def _build_nc():
    nc = bacc.Bacc("TRN2", target_bir_lowering=False, debug=False, num_devices=8)
    io = {}
    io["hsB"] = nc.dram_tensor("hsB", [NCH, 8, 128, 4, 512], f32r, kind="ExternalInput").ap()
    io["wqB"] = nc.dram_tensor("wqB", [8, 128, 4, QD], f32r, kind="ExternalInput").ap()
    io["wkB"] = nc.dram_tensor("wkB", [8, 128, 4, HD], f32r, kind="ExternalInput").ap()
    io["wvB"] = nc.dram_tensor("wvB", [8, 128, 4, HD], f32r, kind="ExternalInput").ap()
    io["wdB"] = nc.dram_tensor("wdB", [32, 128, 4, 128], f32r, kind="ExternalInput").ap()
    io["cosT"] = nc.dram_tensor("cosT", [HD, S], f32, kind="ExternalInput").ap()
    io["sinTs"] = nc.dram_tensor("sinTs", [HD, S], f32, kind="ExternalInput").ap()
    io["mask01"] = nc.dram_tensor("mask01", [128, 640], f32, kind="ExternalInput").ap()
    io["ident"] = nc.dram_tensor("ident", [128, 128], f32, kind="ExternalInput").ap()
    io["ones"] = nc.dram_tensor("ones", [128, 128], f32r, kind="ExternalInput").ap()
    io["outT"] = nc.dram_tensor("outT", [H, NT], f32, kind="ExternalOutput").ap()
    io["kT"] = nc.dram_tensor("kT", [HD, NT], f32, kind="ExternalOutput").ap()
    io["vT"] = nc.dram_tensor("vT", [HD, NT], f32, kind="ExternalOutput").ap()

    with tile.TileContext(nc) as tc:
        _emit(nc, tc, io)
    nc.compile()
    return nc


def _emit(nc, tc, io):
    from contextlib import ExitStack

    ctx = ExitStack()
    with ctx:
        const_p = ctx.enter_context(tc.tile_pool(name="const", bufs=1))
        persist_p = ctx.enter_context(tc.tile_pool(name="persist", bufs=1))
        hs_p = ctx.enter_context(tc.tile_pool(name="hs", bufs=2))
        wq_p = ctx.enter_context(tc.tile_pool(name="wq", bufs=2))
        wkv_p = ctx.enter_context(tc.tile_pool(name="wkv", bufs=2))
        rope_p = ctx.enter_context(tc.tile_pool(name="rope", bufs=2))
        exp_p = ctx.enter_context(tc.tile_pool(name="exp", bufs=2))
        nrm_p = ctx.enter_context(tc.tile_pool(name="nrm", bufs=1))
        stg_p = ctx.enter_context(tc.tile_pool(name="stg", bufs=6))
        wd_p = ctx.enter_context(tc.tile_pool(name="wd", bufs=4))
        osb_p = ctx.enter_context(tc.tile_pool(name="osb", bufs=4))

        # ---- constants / small tables ----
        cos_sb = const_p.tile([HD, S], f32, tag="cos", name="cos")
        nc.sync.dma_start(cos_sb[:], io["cosT"])
        sin_sb = const_p.tile([HD, S], f32, tag="sin", name="sin")
        nc.sync.dma_start(sin_sb[:], io["sinTs"])
        mask_sb = const_p.tile([128, 640], f32, tag="mask", name="mask")
        nc.sync.dma_start(mask_sb[:], io["mask01"])
        ident_sb = const_p.tile([128, 128], f32, tag="ident", name="ident")
        nc.sync.dma_start(ident_sb[:], io["ident"])
        ones_sb = const_p.tile([128, 128], f32r, tag="ones", name="ones")
        nc.sync.dma_start(ones_sb[:], io["ones"])

        # ---- persistent activations ----
        qt = [persist_p.tile([128, NT], f32r, tag=f"qt{j}", name=f"qt{j}") for j in range(GROUP)]
        ktr = persist_p.tile([128, NT], f32r, tag="ktr", name="ktr")
        vnat = [persist_p.tile([128, 128], f32r, tag=f"vn{i}", name=f"vn{i}") for i in range(NT // 128)]
        ctxT = [persist_p.tile([128, NT], f32r, tag=f"cx{j}", name=f"cx{j}") for j in range(GROUP)]

        HB = 4          # h-chunks per batched DMA

        # ================= stage A: QKV projections + RoPE =================
        ps_a_ctx = ExitStack()
        ps_a = ps_a_ctx.enter_context(tc.tile_pool(name="ps_a", bufs=1, space="PSUM"))
        for c in range(NCH):
            n0 = c * 512
            qps = [ps_a.tile([128, 512], f32, tag=f"qps{j}", name=f"qps{j}") for j in range(GROUP)]
            kps = ps_a.tile([128, 512], f32, tag="kps", name="kps")
            vps = ps_a.tile([128, 512], f32, tag="vps", name="vps")
            for hb in range(HCH // HB):
                h0 = hb * HB
                hs_t = hs_p.tile([128, HB, 512], f32r, tag="hs", name="hs")
                nc.sync.dma_start(hs_t[:], io["hsB"][c, hb])
                wq_t = wq_p.tile([128, HB, QD], f32r, tag="wq", name="wq")
                nc.sync.dma_start(wq_t[:], io["wqB"][hb])
                wk_t = wkv_p.tile([128, HB, HD], f32r, tag="wk", name="wk")
                nc.sync.dma_start(wk_t[:], io["wkB"][hb])
                wv_t = wkv_p.tile([128, HB, HD], f32r, tag="wv", name="wv")
                nc.sync.dma_start(wv_t[:], io["wvB"][hb])
                for i in range(HB):
                    h = h0 + i
                    st = (h == 0)
                    sp = (h == HCH - 1)
                    for j in range(GROUP):
                        nc.tensor.matmul(qps[j][:], wq_t[:, i, j * 128:(j + 1) * 128],
                                         hs_t[:, i, :], start=st, stop=sp)
                    nc.tensor.matmul(kps[:], wk_t[:, i, :], hs_t[:, i, :], start=st, stop=sp)
                    nc.tensor.matmul(vps[:], wv_t[:, i, :], hs_t[:, i, :], start=st, stop=sp)

            off = (c % (S // 512)) * 512  # position offset within batch
            cos_sl = cos_sb[:, off:off + 512]
            sin_sl = sin_sb[:, off:off + 512]
            # stage all 6 psums to SBUF fast (split ACT/DVE) so banks free early
            qsb = [stg_p.tile([128, 512], f32, tag="stg", name="stg") for _ in range(GROUP)]
            ksb = stg_p.tile([128, 512], f32, tag="stg", name="stg")
            vt_out = rope_p.tile([128, 512], f32, tag="vstage", name="vstage")
            nc.scalar.copy(qsb[0][:], qps[0][:])
            nc.vector.tensor_copy(qsb[1][:], qps[1][:])
            nc.scalar.copy(qsb[2][:], qps[2][:])
            nc.vector.tensor_copy(qsb[3][:], qps[3][:])
            nc.scalar.copy(ksb[:], kps[:])
            nc.vector.tensor_copy(vt_out[:], vps[:])
            nc.gpsimd.dma_start(io["vT"][:, n0:n0 + 512], vt_out[:])
            # Q RoPE -> qt[j] (f32r)
            for j in range(GROUP):
                rq = rope_p.tile([128, 512], f32, tag="rq", name="rq")
                nc.vector.tensor_copy(rq[0:64, :], qsb[j][64:128, :])
                nc.vector.tensor_copy(rq[64:128, :], qsb[j][0:64, :])
                nc.vector.tensor_mul(rq[:], rq[:], sin_sl)
                qc_t = rope_p.tile([128, 512], f32, tag="qcos", name="qcos")
                nc.vector.tensor_mul(qc_t[:], qsb[j][:], cos_sl)
                nc.vector.tensor_add(qt[j][:, n0:n0 + 512], qc_t[:], rq[:])
            # K RoPE -> kt_out (f32, staged for DMA) + ktr (f32r)
            rk = rope_p.tile([128, 512], f32, tag="rk", name="rk")
            nc.vector.tensor_copy(rk[0:64, :], ksb[64:128, :])
            nc.vector.tensor_copy(rk[64:128, :], ksb[0:64, :])
            nc.vector.tensor_mul(rk[:], rk[:], sin_sl)
            kt_out = rope_p.tile([128, 512], f32, tag="kcos", name="kcos")
            nc.vector.tensor_mul(kt_out[:], ksb[:], cos_sl)
            nc.vector.tensor_add(kt_out[:], kt_out[:], rk[:])
            nc.scalar.copy(ktr[:, n0:n0 + 512], kt_out[:])
            nc.gpsimd.dma_start(io["kT"][:, n0:n0 + 512], kt_out[:])
            # V transpose -> vnat (f32r), 4 tiles of 128
            for i in range(4):
                g = c * 4 + i
                tp = ps_a.tile([128, 128], f32, tag="kps", name="tp")
                nc.tensor.transpose(tp[:], vt_out[:, i * 128:(i + 1) * 128], ident_sb[:])
                nc.vector.tensor_copy(vnat[g][:], tp[:])

        ps_a_ctx.close()
        # ================= stage B: attention per (batch, q-head) =================
        ps_b_ctx = ExitStack()
        ps_b = ps_b_ctx.enter_context(tc.tile_pool(name="ps_b", bufs=1, space="PSUM"))
        for b in range(B):
            base = b * S
            for j in range(GROUP):
                cps = ps_b.tile([128, S], f32, tag="cps", name="cps")    # ctxT_un [hd, q]
                dps = ps_b.tile([128, S], f32, tag="dps", name="dps")    # denom bcast [*, q]
                for t in range(S // 128):                    # key chunks
                    qcd = t // 4                             # q-chunk holding the diagonal
                    sps = ps_b.tile([128, S], f32, tag="sps", name="sps", bufs=2)
                    et = exp_p.tile([128, S], f32r, tag="et", name="et")
                    for qc in range(S // 512):
                        if qc < qcd:
                            continue                          # fully masked
                        q0 = qc * 512
                        nc.tensor.matmul(sps[:, q0:q0 + 512],
                                         ktr[:, base + t * 128:base + (t + 1) * 128],
                                         qt[j][:, base + q0:base + q0 + 512])
                        nc.scalar.activation(et[:, q0:q0 + 512], sps[:, q0:q0 + 512], AF.Exp)
                    # zero the masked region: columns [qcd*512 .. t*128+128)
                    o = t * 128 - qcd * 512
                    nc.vector.tensor_mul(et[:, qcd * 512:t * 128 + 128],
                                         et[:, qcd * 512:t * 128 + 128].bitcast(f32),
                                         mask_sb[:, 512 - o:640])
                    for qc in range(S // 512):
                        if qc < qcd:
                            continue
                        q0 = qc * 512
                        st = (t == 0)
                        sp = (t == (qc * 4 + 3))
                        nc.tensor.matmul(cps[:, q0:q0 + 512], vnat[b * 8 + t][:],
                                         et[:, q0:q0 + 512], start=st, stop=sp)
                        nc.tensor.matmul(dps[:, q0:q0 + 512], ones_sb[:],
                                         et[:, q0:q0 + 512], start=st, stop=sp)
                rec = nrm_p.tile([128, S], f32, tag="rec", name="rec")
                scr = nrm_p.tile([128, S], f32, tag="scr", name="scr")
                nc.vector.reciprocal_approx_accurate(rec[:], dps[:], scr[:])
                nc.vector.tensor_mul(ctxT[j][:, base:base + S], cps[:], rec[:])

        ps_b_ctx.close()
        # ================= stage C: dense partial out.T = wd_d @ ctx_d.T =============
        ps_c = ctx.enter_context(tc.tile_pool(name="ps_c", bufs=1, space="PSUM"))
        for cp in range(2):                                  # batch halves (n chunks 2cp, 2cp+1)
            for ho in range(H // 128):
                ops = [ps_c.tile([128, 512], f32, tag=f"ops{i}", name=f"ops{i}", bufs=2)
                       for i in range(2)]
                wd_t = wd_p.tile([128, GROUP, 128], f32r, tag="wd", name="wd")
                nc.sync.dma_start(wd_t[:], io["wdB"][ho])
                for ci in range(GROUP):
                    for i in range(2):
                        n0 = (2 * cp + i) * 512
                        nc.tensor.matmul(ops[i][:], wd_t[:, ci, :], ctxT[ci][:, n0:n0 + 512],
                                         start=(ci == 0), stop=(ci == GROUP - 1))
                for i in range(2):
                    n0 = (2 * cp + i) * 512
                    ot = osb_p.tile([128, 512], f32, tag="ot", name="ot")
                    if i == 0:
                        nc.scalar.copy(ot[:], ops[i][:])
                    else:
                        nc.vector.tensor_copy(ot[:], ops[i][:])
                    nc.gpsimd.dma_start(io["outT"][ho * 128:(ho + 1) * 128, n0:n0 + 512], ot[:])


def _host_inputs(hidden_states, cos, sin, wq, wk, wv, wd):
    """Build the 8 per-core input maps (all numpy, fp32)."""
    hs = np.asarray(hidden_states, dtype=np.float32).reshape(NT, H)
    hsT = np.ascontiguousarray(hs.T)
    scale = np.float32(1.0 / math.sqrt(HD))
    wq = np.asarray(wq, dtype=np.float32)
    wk = np.asarray(wk, dtype=np.float32)
    wv = np.asarray(wv, dtype=np.float32)
    wd = np.asarray(wd, dtype=np.float32)
    wqsT = (wq * scale).T                              # [H, NH*HD]
    wkT = wk.T                                         # [H, NKV*HD]
    wvT = wv.T
    wdT = wd.T                                         # [H_in(ctx), H_out]
    # blocked DMA layouts: every on-chip DMA reads one contiguous block
    hsB = np.ascontiguousarray(
        hsT.reshape(8, 4, 128, NCH, 512).transpose(3, 0, 2, 1, 4))

    cosT = np.ascontiguousarray(np.asarray(cos, dtype=np.float32)[0, 0].T)  # [HD, S]
    sinT = np.ascontiguousarray(np.asarray(sin, dtype=np.float32)[0, 0].T)
    sinTs = sinT.copy()
    sinTs[0:HD // 2] = -sinT[0:HD // 2]

    mask01 = np.zeros((128, 640), np.float32)
    tri = (np.arange(128)[:, None] <= np.arange(128)[None, :])
    mask01[:, 512:640] = tri.astype(np.float32)
    ident = np.eye(128, dtype=np.float32)
    ones = np.ones((128, 128), np.float32)

    in_maps = []
    for d in range(8):
        wq_d = wqsT[:, d * QD:(d + 1) * QD]            # [H, 512]
        wk_d = wkT[:, d * HD:(d + 1) * HD]
        wv_d = wvT[:, d * HD:(d + 1) * HD]
        wd_d = wdT[d * QD:(d + 1) * QD, :]             # [512, H]
        in_maps.append({
            "hsB": hsB,
            "wqB": np.ascontiguousarray(wq_d.reshape(8, 4, 128, QD).transpose(0, 2, 1, 3)),
            "wkB": np.ascontiguousarray(wk_d.reshape(8, 4, 128, HD).transpose(0, 2, 1, 3)),
            "wvB": np.ascontiguousarray(wv_d.reshape(8, 4, 128, HD).transpose(0, 2, 1, 3)),
            "wdB": np.ascontiguousarray(wd_d.reshape(4, 128, 32, 128).transpose(2, 1, 0, 3)),
            "cosT": cosT,
            "sinTs": sinTs,
            "mask01": mask01,
            "ident": ident,
            "ones": ones,
        })
    return in_maps


def _gather(results):
    outT = np.zeros((H, NT), np.float64)
    for d in range(8):
        outT += results[d]["outT"]
    out = outT.T.astype(np.float32).reshape(B, S, H)
    k = np.stack([results[d]["kT"].reshape(HD, B, S).transpose(1, 2, 0)
                  for d in range(8)], axis=1)             # [B, NKV, S, HD]
    v = np.stack([results[d]["vT"].reshape(HD, B, S).transpose(1, 2, 0)
                  for d in range(8)], axis=1)
    return out, np.ascontiguousarray(k), np.ascontiguousarray(v)


def _run(in_maps, **kwargs):
    if "nc" not in _CACHE:
        _CACHE["nc"] = _build_nc()
    return run_bass_kernel_spmd(_CACHE["nc"], in_maps, list(range(8)), **kwargs)


def kernel(hidden_states, cos, sin, wq, wk, wv, wd):
    in_maps = _host_inputs(hidden_states, cos, sin, wq, wk, wv, wd)
    res = _run(in_maps)
    return _gather(res.results)
